# revision 1
# baseline (speedup 1.0000x reference)
"""CrossAttentionS2T (attn_all_frame=True) as a Bass/Tile kernel on 8 trn2 cores.

Strategy: data-parallel over batch B=8 -> one batch element per NeuronCore.
Per core, all activations live in transposed [feature, token] layout so every
matmul contracts over the partition axis at full 128-wide PE utilization:

  q_in.T [768,1568]   = t_x slice.T + pos (device add)
  s.T    [768, 784]   = s_x slice.T + pos (device add)
  q.T  = (0.125*Wq) @ q_in.T + 0.125*qb      (scale folded into weights: exact)
  k.T  = Wk @ s.T + kb ; v (natural) = s.T.T @ Wv.T + vb
  scores.T[k,q] = k_h.T^T-contraction -> exp (no max-sub; scores are O(1))
  [o_unnorm.T ; denom] = [v_h | 1]^T @ probs.T   (ones column => softmax denom)
  o.T = o_unnorm.T * bcast(1/denom)
  out.T = Wproj @ o.T + pb -> PE-transpose -> natural [1568,768] -> DMA out

Matmul inputs are bitcast to float32r (full fp32 data, 1 cycle/row for moving
free dim >= 256 on trn2 vs 4 cycles/row for plain fp32).
"""

import math
import os
from contextlib import ExitStack

import numpy as np

import concourse.bass as bass
import concourse.mybir as mybir
import concourse.tile as tile
from concourse.bass import ds, ts
from concourse.masks import make_identity

F32 = mybir.dt.float32
F32R = mybir.dt.float32r
AF = mybir.ActivationFunctionType

# problem dims (hardcoded per contract)
B, SPEC, T = 8, 4, 8
AP_, VP, DIM = 196, 196, 768
NH, HD = 12, 64
SCALE = HD ** -0.5
NQ = VP * T          # 1568 q tokens per batch
NK = AP_ * SPEC      # 784 kv tokens per batch
DC = DIM // 128      # 6 contraction chunks
QT, NQT = 392, 4     # q-token tile (moving free dim)
KB, NKB = 112, 7     # k-token block (scores.T partition dim)
VW, NVW = 384, 2     # v feature tile for natural-layout V projection
NCORES = 8


def _r(ap):
    return ap.bitcast(F32R)


def _emit(ctx, tc, outs, ins):
    nc = tc.nc
    (txT, sxT, posqT, possT, wqT, wkT, wvT, wpT, qb2, kb2, pb2, vbb, ones1) = ins
    (out_d,) = outs

    const = ctx.enter_context(tc.tile_pool(name="const", bufs=1))
    ident = const.tile([128, 128], F32)
    make_identity(nc, ident)
    qb_t = const.tile([128, DC], F32)
    kb_t = const.tile([128, DC], F32)
    pb_t = const.tile([128, DC], F32)
    vbb_t = const.tile([128, DIM], F32)
    nc.sync.dma_start(qb_t[:], qb2[:])
    nc.sync.dma_start(kb_t[:], kb2[:])
    nc.sync.dma_start(pb_t[:], pb2[:])
    nc.sync.dma_start(vbb_t[:], vbb[:])
    ones_t = const.tile([1, HD], F32)
    nc.sync.dma_start(_r(ones_t[:]), _r(ones1[:]))

    # persistent activations
    pers = ctx.enter_context(tc.tile_pool(name="pers", bufs=1))
    oT = [pers.tile([128, NQ], F32, name=f"oT{c}", tag=f"oT{c}") for c in range(DC)]
    kTt = [pers.tile([128, NK], F32, name=f"kT{c}", tag=f"kT{c}") for c in range(DC)]
    v_t = [pers.tile([KB, NH * (HD + 1)], F32, name=f"v{j}", tag=f"v{j}")
           for j in range(NKB)]
    qTt = [pers.tile([128, NQ], F32, name=f"qT{c}", tag=f"qT{c}") for c in range(DC)]
    wp_t = [pers.tile([128, DIM], F32, name=f"wp{c}", tag=f"wp{c}") for c in range(DC)]
    for c in range(DC):
        nc.sync.dma_start(_r(wp_t[c][:]), _r(wpT[ts(c, 128), :]))

    # PSUM pools: proj (3 banks) + attention qk (3) + o (2) = 8 banks total
    psA = ctx.enter_context(tc.tile_pool(name="psA", bufs=2, space="PSUM"))
    psB = ctx.enter_context(tc.tile_pool(name="psB", bufs=1, space="PSUM"))

    # ---- phase A: s.T build + KV projections ----
    with tc.tile_pool(name="phA", bufs=1) as phA:
        wk_t = [phA.tile([128, DIM], F32, name=f"wk{c}", tag=f"wk{c}")
                for c in range(DC)]
        wv_t = [phA.tile([128, DIM], F32, name=f"wv{c}", tag=f"wv{c}")
                for c in range(DC)]
        sT = [phA.tile([128, NK], F32, name=f"sT{c}", tag=f"sT{c}")
              for c in range(DC)]
        for c in range(DC):
            nc.sync.dma_start(_r(wk_t[c][:]), _r(wkT[ts(c, 128), :]))
            nc.sync.dma_start(_r(wv_t[c][:]), _r(wvT[ts(c, 128), :]))
        for c in range(DC):
            sx_t = phA.tile([128, NK], F32, name="sx_t", tag="ldA", bufs=2)
            nc.sync.dma_start(sx_t[:], sxT[ts(c, 128), :])
            ps_t = phA.tile([128, NK], F32, name="ps_t", tag="ldB", bufs=2)
            nc.sync.dma_start(ps_t[:], possT[ts(c, 128), :])
            nc.vector.tensor_add(_r(sT[c][:]), sx_t[:], ps_t[:])

        # K projection, transposed output layout [kfeat, ktok]
        for f in range(DC):
            for kt in range(2):
                ps = psA.tile([128, QT], F32, name="ps_k", tag="proj")
                for c in range(DC):
                    nc.tensor.matmul(
                        ps[:], _r(wk_t[c][:, ts(f, 128)]),
                        _r(sT[c][:, ts(kt, QT)]),
                        start=(c == 0), stop=(c == DC - 1))
                nc.scalar.activation(_r(kTt[f][:, ts(kt, QT)]), ps[:], AF.Identity,
                                     bias=kb_t[:, ds(f, 1)])

        # V projection, natural layout [ktok, vfeat], +1s column per head
        # (memset is not encodable with an f32r output; round via tensor_copy)
        vinit = phA.tile([KB, NH * (HD + 1)], F32, name="vinit", tag="vinit")
        nc.vector.memset(vinit[:], 1.0)
        for j in range(NKB):
            nc.vector.tensor_copy(_r(v_t[j][:]), vinit[:])
            for w in range(NVW):
                ps = psA.tile([KB, VW], F32, name="ps_v", tag="proj")
                for c in range(DC):
                    nc.tensor.matmul(
                        ps[:], _r(sT[c][:, ts(j, KB)]),
                        _r(wv_t[c][:, ts(w, VW)]),
                        start=(c == 0), stop=(c == DC - 1))
                for hh in range(6):
                    h = w * 6 + hh
                    nc.vector.tensor_add(
                        _r(v_t[j][:, ds(h * (HD + 1), HD)]),
                        ps[:, ts(hh, HD)],
                        vbb_t[0:KB, ds(w * VW + hh * HD, HD)])

    # ---- phase B: q_in.T build + Q projection (streamed per q-tile) ----
    with tc.tile_pool(name="phB", bufs=1) as phB:
        wq_t = [phB.tile([128, DIM], F32, name=f"wq{c}", tag=f"wq{c}")
                for c in range(DC)]
        for c in range(DC):
            nc.sync.dma_start(_r(wq_t[c][:]), _r(wqT[ts(c, 128), :]))
        for qt in range(NQT):
            qins = []
            for c in range(DC):
                tx_t = phB.tile([128, QT], F32, name="tx_t", tag="ldq", bufs=3)
                nc.gpsimd.dma_start(tx_t[:], txT[ts(c, 128), ts(qt, QT)])
                pq_t = phB.tile([128, QT], F32, name="pq_t", tag="ldp", bufs=3)
                nc.gpsimd.dma_start(pq_t[:], posqT[ts(c, 128), ts(qt, QT)])
                qin_c = phB.tile([128, QT], F32, name="qin", tag="qin", bufs=2 * DC)
                nc.vector.tensor_add(_r(qin_c[:]), tx_t[:], pq_t[:])
                qins.append(qin_c)
            for f in range(DC):
                ps = psA.tile([128, QT], F32, name="ps_q", tag="proj")
                for c in range(DC):
                    nc.tensor.matmul(
                        ps[:], _r(wq_t[c][:, ts(f, 128)]), _r(qins[c][:]),
                        start=(c == 0), stop=(c == DC - 1))
                nc.scalar.activation(_r(qTt[f][:, ts(qt, QT)]), ps[:], AF.Identity,
                                     bias=qb_t[:, ds(f, 1)])

    # ---- phase C: attention ----
    with tc.tile_pool(name="phC", bufs=1) as phC:
        for h in range(NH):
            ch, off = h // 2, (h % 2) * HD
            for qt in range(NQT):
                q_ap = qTt[ch][ds(off, HD), ts(qt, QT)]
                o_ps = psB.tile([HD + 1, QT], F32, name="o_ps", tag="o", bufs=3)
                probs = []
                for j in range(NKB):
                    s_ps = psB.tile([KB, QT], F32, name="s_ps", tag="qk", bufs=3)
                    nc.tensor.matmul(s_ps[:],
                                     _r(kTt[ch][ds(off, HD), ts(j, KB)]),
                                     _r(q_ap), start=True, stop=True)
                    p_t = phC.tile([KB, QT], F32, name="p_t", tag="probs", bufs=9)
                    nc.scalar.activation(_r(p_t[:]), s_ps[:], AF.Exp)
                    probs.append(p_t)
                for j in range(NKB):
                    nc.tensor.matmul(o_ps[:],
                                     _r(v_t[j][:, ds(h * (HD + 1), HD + 1)]),
                                     _r(probs[j][:]),
                                     start=(j == 0), stop=(j == NKB - 1))
                r1 = phC.tile([1, QT], F32R, name="r1", tag="r1", bufs=2)
                with nc.allow_low_precision(reason="f32r recip for bcast"):
                    nc.vector.reciprocal(r1[:], o_ps[ds(HD, 1), :])
                rb_ps = psB.tile([HD, QT], F32, name="rb_ps", tag="qk", bufs=3)
                nc.tensor.matmul(rb_ps[:], _r(ones_t[:]), r1[:],
                                 start=True, stop=True)
                rb = phC.tile([HD, QT], F32, name="rb", tag="rb", bufs=2)
                nc.vector.tensor_copy(rb[:], rb_ps[:])
                nc.vector.tensor_mul(_r(oT[ch][ds(off, HD), ts(qt, QT)]),
                                     o_ps[0:HD, :], rb[:])

    # ---- phase D: output projection + transpose to natural + DMA out ----
    with tc.tile_pool(name="phD", bufs=1) as phD:
        outT = [phD.tile([128, NQ], F32, name=f"outT{c}", tag=f"outT{c}")
                for c in range(DC)]
        for f in range(DC):
            for qt in range(NQT):
                ps = psA.tile([128, QT], F32, name="ps_o", tag="proj")
                for c in range(DC):
                    nc.tensor.matmul(
                        ps[:], _r(wp_t[c][:, ts(f, 128)]),
                        _r(oT[c][:, ts(qt, QT)]),
                        start=(c == 0), stop=(c == DC - 1))
                nc.scalar.activation(outT[f][:, ts(qt, QT)], ps[:], AF.Identity,
                                     bias=pb_t[:, ds(f, 1)])
        nblk = math.ceil(NQ / 128)  # 13 blocks: 12x128 + 32
        for qb in range(nblk):
            qw = min(128, NQ - qb * 128)
            o_nat = phD.tile([128, DIM], F32, name="o_nat", tag="onat", bufs=2)
            for f in range(DC):
                tp = psA.tile([128, 128], F32, name="tp", tag="proj")
                nc.tensor.transpose(tp[0:qw, :], outT[f][:, ds(qb * 128, qw)],
                                    ident[:])
                nc.vector.tensor_copy(o_nat[0:qw, ts(f, 128)], tp[0:qw, :])
            nc.sync.dma_start(out_d[ds(qb * 128, qw), :], o_nat[0:qw, :])


def build_program():
    from concourse import bacc
    from concourse.compiler_utils import get_compiler_flags, set_compiler_flags
    flags = [f.replace("--enable-ldw-opt=false", "--enable-ldw-opt=true")
             for f in get_compiler_flags()]
    set_compiler_flags(flags)
    nc = bacc.Bacc("TRN2", target_bir_lowering=False, debug=False,
                   num_devices=NCORES)
    mk = lambda name, shape, out=False: nc.dram_tensor(
        name, shape, F32, kind="ExternalOutput" if out else "ExternalInput").ap()
    ins = [
        mk("txT", [DIM, NQ]), mk("sxT", [DIM, NK]),
        mk("posqT", [DIM, NQ]), mk("possT", [DIM, NK]),
        mk("wqT", [DIM, DIM]), mk("wkT", [DIM, DIM]),
        mk("wvT", [DIM, DIM]), mk("wpT", [DIM, DIM]),
        mk("qb2", [128, DC]), mk("kb2", [128, DC]), mk("pb2", [128, DC]),
        mk("vbb", [128, DIM]), mk("ones1", [1, HD]),
    ]
    outs = [mk("out", [NQ, DIM], out=True)]
    with tile.TileContext(nc) as tc:
        with ExitStack() as ctx:
            _emit(ctx, tc, outs, ins)
    nc.compile()
    return nc


def host_prep(inputs):
    """Host-side layout marshalling: slice per core, transpose to
    [feature, token], fold the attention scale into Wq (exact: 0.125 = 2^-3),
    pre-broadcast positional sums and biases."""
    f32 = np.float32
    g = {k: np.asarray(v, dtype=f32) for k, v in inputs.items()}
    t_pat = g["t_x"][1:]                      # (VP, B*T, D)
    s_x = g["s_x"]                            # (AP, B*SPEC, D)

    posq = (g["vmae_space_pos"][:, None, :] + g["vmae_temporal_pos"][None, :, :])
    posq = np.ascontiguousarray(posq.reshape(NQ, DIM).T)          # (D, NQ)
    poss = (g["clip_space_pos"][:, None, :] + g["clip_temporal_pos"][None, :, :])
    poss = np.ascontiguousarray(poss.reshape(NK, DIM).T)          # (D, NK)

    wqT = np.ascontiguousarray((SCALE * g["Wq"]).T)
    wkT = np.ascontiguousarray(g["Wkv"][:DIM].T)
    wvT = np.ascontiguousarray(g["Wkv"][DIM:].T)
    wpT = np.ascontiguousarray(g["Wproj"].T)
    qb2 = np.ascontiguousarray((SCALE * g["q_bias"]).reshape(DC, 128).T)
    kb2 = np.ascontiguousarray(g["kv_bias"][:DIM].reshape(DC, 128).T)
    pb2 = np.ascontiguousarray(g["proj_bias"].reshape(DC, 128).T)
    vbb = np.ascontiguousarray(np.tile(g["kv_bias"][DIM:], (128, 1)))

    shared = dict(posqT=posq, possT=poss, wqT=wqT, wkT=wkT, wvT=wvT, wpT=wpT,
                  qb2=qb2, kb2=kb2, pb2=pb2, vbb=vbb,
                  ones1=np.ones((1, HD), dtype=f32))
    in_maps = []
    for b in range(B):
        txT = np.ascontiguousarray(
            t_pat[:, b * T:(b + 1) * T, :].reshape(NQ, DIM).T)
        sxT = np.ascontiguousarray(
            s_x[:, b * SPEC:(b + 1) * SPEC, :].reshape(NK, DIM).T)
        in_maps.append(dict(txT=txT, sxT=sxT, **shared))
    return in_maps


def host_finish(results, t_x):
    o = np.stack([results[b]["out"] for b in range(B)])   # (B, NQ, D)
    o = o.reshape(B, VP, T, DIM).transpose(1, 0, 2, 3).reshape(VP, B * T, DIM)
    return np.concatenate([np.asarray(t_x, dtype=np.float32)[0:1], o], axis=0)


_NC = None


def kernel(**inputs):
    global _NC
    from concourse.bass_utils import run_bass_kernel_spmd
    if _NC is None:
        _NC = build_program()
    in_maps = host_prep(inputs)
    res = run_bass_kernel_spmd(_NC, in_maps, list(range(NCORES)))
    return host_finish(res.results, inputs["t_x"])



# revision 11
# speedup vs baseline: 1.5213x; 1.5213x over previous
"""CrossAttentionS2T (attn_all_frame=True) as a Bass/Tile kernel on 8 trn2 cores.

Strategy: data-parallel over batch B=8 -> one batch element per NeuronCore.
Host precomputes the positional adds and all transposes; the device does the
four GEMMs + attention in a single software-pipelined pass per q-tile:

  qT[f,qt]   = (0.125*Wq) @ qinT[:,qt]               (scale folded into Wq)
  kT         = Wk @ ssT ; v (natural, bf16, +ones col per head) = ssT.T @ Wv.T
  scores.T   = k_h.T^T-contraction, QK for head pairs packed into PE row
               groups (rows 0-63 / 64-127) so two heads stream concurrently
  probs      = exp(scores) via multi-bank ACT instructions (4 PSUM banks per
               activation), written bf16
  [o.T;den]  = [v_h | 1]^T @ probs.T   (ones column => softmax denominator)
  o.T        = o.T * bcast(1/den)      (DVE reciprocal_approx_fast + gpsimd
               partition_broadcast + DVE multiply)
  outT       = Wp @ o.T + pb -> DMA out as [768, 1568]; host transposes.

Matmul inputs are f32 bitcast to float32r (1 cycle/row for moving free dim
>= 256) except AV which runs bf16 x bf16.
"""

import math
import os
from contextlib import ExitStack

import numpy as np

import concourse.bass as bass
import concourse.mybir as mybir
import concourse.tile as tile
from concourse.bass import ds, ts

F32 = mybir.dt.float32
F32R = mybir.dt.float32r
BF16 = mybir.dt.bfloat16
AF = mybir.ActivationFunctionType

# problem dims (hardcoded per contract)
B, SPEC, T = 8, 4, 8
AP_, VP, DIM = 196, 196, 768
NH, HD = 12, 64
SCALE = HD ** -0.5
NQ = VP * T          # 1568 q tokens per batch
NK = AP_ * SPEC      # 784 kv tokens per batch
DC = DIM // 128      # 6 contraction chunks
QT, NQT = 392, 4     # q-token tile (moving free dim)
KB, NKB = 112, 7     # k-token block (scores.T partition dim)
VW, NVW = 384, 2     # v feature tile for natural-layout V projection
NCORES = 8

# exp group structure per head-pair: 14 (head, j) blocks in groups of
# (4, 1, 4, 1, 4) PSUM banks; seq s = 2*j + (0 for even head, 1 for odd).
GRP_SIZES = (4, 1, 4, 1, 4)
GRP_START = (0, 4, 5, 9, 10)


def _grp_of(s):
    for g in range(len(GRP_SIZES) - 1, -1, -1):
        if s >= GRP_START[g]:
            return g, s - GRP_START[g]
    raise AssertionError


def _r(ap):
    return ap.bitcast(F32R)


def _emit(ctx, tc, outs, ins):
    nc = tc.nc
    (qinT_d, ssT_d, wqT, wkT, wvT, wpT, qb2, kb2, pb2, vbb, ones1) = ins
    out_d = outs[0]
    dbg = outs[1:] if len(outs) > 1 else None

    const = ctx.enter_context(tc.tile_pool(name="const", bufs=1))
    qb_t = const.tile([128, DC], F32)
    kb_t = const.tile([128, DC], F32)
    pb_t = const.tile([128, DC], F32)
    vbb_t = const.tile([128, DIM], F32)
    nc.sync.dma_start(qb_t[:], qb2[:])
    nc.sync.dma_start(kb_t[:], kb2[:])
    nc.sync.dma_start(pb_t[:], pb2[:])
    nc.sync.dma_start(vbb_t[:], vbb[:])
    ones_t = const.tile([1, HD], F32)
    nc.sync.dma_start(ones_t[:], ones1[:])

    # persistent pool: weights for Q/out proj, K/V activations, qin/qT streams
    pers = ctx.enter_context(tc.tile_pool(name="pers", bufs=1))
    wq_t = [pers.tile([128, DIM], F32, name=f"wq{c}", tag=f"wq{c}")
            for c in range(DC)]
    wp_t = [pers.tile([128, DIM], F32, name=f"wp{c}", tag=f"wp{c}")
            for c in range(DC)]
    kTt = [pers.tile([128, NK], F32, name=f"kT{c}", tag=f"kT{c}")
           for c in range(DC)]
    v_t = [pers.tile([KB, NH * (HD + 1)], BF16, name=f"v{j}", tag=f"v{j}")
           for j in range(NKB)]

    # PSUM: s0 (4 banks) + s1 (1) + o (2) + proj (1) = 8 banks
    psum = ctx.enter_context(tc.tile_pool(name="psum", bufs=1, space="PSUM"))

    def psum_tile(shape, name, tag, bufs=1):
        return psum.tile(shape, F32, name=name, tag=tag, bufs=bufs)

    # ---- phase A: K/V projections from host-prepped ssT ----
    with tc.tile_pool(name="phA", bufs=1) as phA:
        wk_t = [phA.tile([128, DIM], F32, name=f"wk{c}", tag=f"wk{c}")
                for c in range(DC)]
        wv_t = [phA.tile([128, DIM], F32, name=f"wv{c}", tag=f"wv{c}")
                for c in range(DC)]
        sT = [phA.tile([128, NK], F32, name=f"sT{c}", tag=f"sT{c}")
              for c in range(DC)]
        for c in range(DC):
            nc.sync.dma_start(_r(wk_t[c][:]), _r(wkT[ts(c, 128), :]))
            nc.sync.dma_start(_r(sT[c][:]), _r(ssT_d[ts(c, 128), :]))
            nc.sync.dma_start(_r(wv_t[c][:]), _r(wvT[ts(c, 128), :]))
        for c in range(DC):
            nc.sync.dma_start(_r(wq_t[c][:]), _r(wqT[ts(c, 128), :]))
            nc.sync.dma_start(_r(wp_t[c][:]), _r(wpT[ts(c, 128), :]))

        # during phase A the attention banks are free: cycle 4 psum slots
        pha_slots = [("s0", [128, 4, 512], 1), ("s1", [128, 512], 1),
                     ("o", [128, 512], 2), ("proj", [128, 512], 1)]
        slot_i = [0]

        def pha_psum():
            tag, shape, bufs = pha_slots[slot_i[0] % len(pha_slots)]
            slot_i[0] += 1
            t = psum_tile(shape, "psA", tag, bufs=bufs)
            return t[:, 0, :] if len(shape) == 3 else t

        # K projection, transposed output layout [kfeat, ktok]
        for f in range(DC):
            for kt in range(2):
                ps = pha_psum()
                for c in range(DC):
                    nc.tensor.matmul(
                        ps[0:128, 0:QT], _r(wk_t[c][:, ts(f, 128)]),
                        _r(sT[c][:, ts(kt, QT)]),
                        start=(c == 0), stop=(c == DC - 1))
                nc.vector.tensor_scalar_add(
                    _r(kTt[f][:, ts(kt, QT)]), ps[0:128, 0:QT],
                    kb_t[:, ds(f, 1)])
        if dbg is not None:
            for f in range(DC):
                nc.sync.dma_start(dbg[0][ts(f, 128), :], kTt[f][:])

        # V projection, natural layout [ktok, vfeat] bf16, +1s col per head
        for j in range(NKB):
            v3 = v_t[j].rearrange("p (h e) -> p h e", e=HD + 1)
            nc.vector.memset(v3[:, :, ds(HD, 1)], 1.0)
            for w in range(NVW):
                ps = pha_psum()
                for c in range(DC):
                    nc.tensor.matmul(
                        ps[0:KB, 0:VW], _r(sT[c][:, ts(j, KB)]),
                        _r(wv_t[c][:, ts(w, VW)]),
                        start=(c == 0), stop=(c == DC - 1))
                nc.vector.tensor_add(
                    v3[:, ds(w * 6, 6), 0:HD],
                    ps[0:KB, 0:VW].rearrange("p (h e) -> p h e", e=HD),
                    vbb_t[0:KB, ts(w, VW)].rearrange("p (h e) -> p h e", e=HD))

    # ---- main pipeline: per q-tile Qproj -> attention -> out proj ----
    main = ctx.enter_context(tc.tile_pool(name="main", bufs=1))
    for qt in range(NQT):
        # Q projection (host folded scale+pos into qinT/Wq)
        qins = []
        for c in range(DC):
            qin_c = pers.tile([128, QT], F32, name="qin", tag="qin", bufs=12)
            nc.gpsimd.dma_start(_r(qin_c[:]), _r(qinT_d[ts(c, 128), ts(qt, QT)]))
            qins.append(qin_c)
        qTt = []
        for f in range(DC):
            ps = psum_tile([128, 512], "ps_q", "proj")
            for c in range(DC):
                nc.tensor.matmul(
                    ps[0:128, 0:QT], _r(wq_t[c][:, ts(f, 128)]),
                    _r(qins[c][:]),
                    start=(c == 0), stop=(c == DC - 1))
            qT_f = pers.tile([128, QT], F32, name="qT", tag="qT", bufs=12)
            nc.vector.tensor_scalar_add(_r(qT_f[:]), ps[0:128, 0:QT],
                                        qb_t[:, ds(f, 1)])
            if dbg is not None and qt == 0:
                nc.sync.dma_start(dbg[1][ts(f, 128), :], qT_f[:])
            qTt.append(qT_f)

        oT = [main.tile([128, QT], F32, name=f"oT{c}", tag=f"oT{c}", bufs=2)
              for c in range(DC)]

        for ch in range(DC):
            # QK for head pair (2ch, 2ch+1): alternate row groups 0/64
            grp_tiles = [None] * len(GRP_SIZES)
            probs = [None] * len(GRP_SIZES)
            for g, gsz in enumerate(GRP_SIZES):
                if gsz == 4:
                    st = psum_tile([128, 4, 512], "s_ps", "s0")
                else:
                    st = psum_tile([128, 512], "s_ps1", "s1")
                grp_tiles[g] = st
                for slot in range(gsz):
                    s = GRP_START[g] + slot
                    par, j = (s % 2) * HD, s // 2
                    dst = (st[0:KB, slot, 0:QT] if gsz == 4
                           else st[0:KB, 0:QT])
                    nc.tensor.matmul(
                        dst, _r(kTt[ch][ds(par, HD), ts(j, KB)]),
                        _r(qTt[ch][ds(par, HD), :]), start=True, stop=True)
                # exp the whole group in one ACT instruction -> bf16 probs
                if gsz == 4:
                    p_t = main.tile([KB, 4, QT], BF16, name="pr4", tag="pr4",
                                    bufs=6)
                    nc.scalar.activation(p_t[0:KB, :, :],
                                         st[0:KB, :, 0:QT], AF.Exp)
                else:
                    p_t = main.tile([KB, QT], BF16, name="pr1", tag="pr1",
                                    bufs=4)
                    nc.scalar.activation(p_t[0:KB, :], st[0:KB, 0:QT], AF.Exp)
                probs[g] = p_t
                if dbg is not None and qt == 0 and ch == 0:
                    w0 = GRP_START[g]
                    if gsz == 4:
                        nc.sync.dma_start(dbg[2][:, ds(w0, 4), :],
                                          p_t[0:KB, :, :])
                    else:
                        nc.sync.dma_start(dbg[2][:, w0, :], p_t[0:KB, :])

            def pslice(h_odd, j):
                g, slot = _grp_of(2 * j + h_odd)
                p = probs[g]
                return p[0:KB, slot, :] if GRP_SIZES[g] == 4 else p[0:KB, :]

            for h_odd in range(2):
                h = 2 * ch + h_odd
                o_ps = psum_tile([128, 512], "o_ps", "o", bufs=2)
                for j in range(NKB):
                    nc.tensor.matmul(
                        o_ps[0:HD + 1, 0:QT],
                        v_t[j][:, ds(h * (HD + 1), HD + 1)],
                        pslice(h_odd, j), start=(j == 0), stop=(j == NKB - 1))
                den_sb = main.tile([1, QT], F32, name="den_sb",
                                   tag="den", bufs=2)
                nc.vector.tensor_copy(den_sb[:], o_ps[ds(HD, 1), 0:QT])
                r1 = main.tile([1, QT], F32, name="r1", tag="r1", bufs=2)
                nc.vector.reciprocal_approx_fast(r1[:], den_sb[:])
                rb_ps = psum_tile([128, 512], "rb_ps", "proj")
                nc.tensor.matmul(rb_ps[0:HD, 0:QT], ones_t[:], r1[:],
                                 start=True, stop=True)
                rb = main.tile([HD, QT], F32, name="rb", tag="rb", bufs=2)
                nc.vector.tensor_copy(rb[:], rb_ps[0:HD, 0:QT])
                if dbg is not None and qt == 0 and ch == 0 and h_odd == 0:
                    nc.sync.dma_start(dbg[5][:], r1[:])
                    nc.sync.dma_start(dbg[6][:], rb[:])
                    nc.sync.dma_start(dbg[7][:], den_sb[:])
                    ou_sb = main.tile([HD, QT], F32, name="ou_sb", tag="ou")
                    nc.vector.tensor_copy(ou_sb[:], o_ps[0:HD, 0:QT])
                    nc.sync.dma_start(dbg[8][:], ou_sb[:])
                nc.vector.tensor_mul(_r(oT[ch][ds(h_odd * HD, HD), :]),
                                     o_ps[0:HD, 0:QT], rb[:])

        if dbg is not None and qt == 0:
            for c in range(DC):
                nc.sync.dma_start(dbg[3][ts(c, 128), :], oT[c][:])
            for j in range(NKB):
                nc.sync.dma_start(dbg[4][ds(j * KB, KB), :], v_t[j][:])

        # output projection, transposed layout [feat, qtok] -> DRAM
        for f in range(DC):
            ps = psum_tile([128, 512], "ps_o", "proj")
            for c in range(DC):
                nc.tensor.matmul(
                    ps[0:128, 0:QT], _r(wp_t[c][:, ts(f, 128)]),
                    _r(oT[c][:]),
                    start=(c == 0), stop=(c == DC - 1))
            outT_f = main.tile([128, QT], F32, name="outT", tag="outT", bufs=4)
            nc.vector.tensor_scalar_add(outT_f[:], ps[0:128, 0:QT],
                                        pb_t[:, ds(f, 1)])
            nc.sync.dma_start(out_d[ts(f, 128), ts(qt, QT)], outT_f[:])


def build_program():
    from concourse import bacc
    from concourse.compiler_utils import get_compiler_flags, set_compiler_flags
    flags = [f.replace("--enable-ldw-opt=false", "--enable-ldw-opt=true")
             for f in get_compiler_flags()]
    set_compiler_flags(flags)
    nc = bacc.Bacc("TRN2", target_bir_lowering=False, debug=False,
                   num_devices=NCORES)
    mk = lambda name, shape, out=False: nc.dram_tensor(
        name, shape, F32, kind="ExternalOutput" if out else "ExternalInput").ap()
    ins = [
        mk("qinT", [DIM, NQ]), mk("ssT", [DIM, NK]),
        mk("wqT", [DIM, DIM]), mk("wkT", [DIM, DIM]),
        mk("wvT", [DIM, DIM]), mk("wpT", [DIM, DIM]),
        mk("qb2", [128, DC]), mk("kb2", [128, DC]), mk("pb2", [128, DC]),
        mk("vbb", [128, DIM]), mk("ones1", [1, HD]),
    ]
    outs = [mk("out", [DIM, NQ], out=True)]
    if os.environ.get("KDBG"):
        outs.append(mk("dbg_kT", [DIM, NK], out=True))
        outs.append(mk("dbg_qT", [DIM, QT], out=True))
        dbp = nc.dram_tensor("dbg_probs", [KB, 14, QT], BF16,
                             kind="ExternalOutput").ap()
        outs.append(dbp)
        outs.append(mk("dbg_oT", [DIM, QT], out=True))
        dbv = nc.dram_tensor("dbg_v", [NKB * KB, NH * (HD + 1)], BF16,
                             kind="ExternalOutput").ap()
        outs.append(dbv)
        outs.append(mk("dbg_r1", [1, QT], out=True))
        outs.append(mk("dbg_rb", [HD, QT], out=True))
        outs.append(mk("dbg_den", [1, QT], out=True))
        outs.append(mk("dbg_ou", [HD, QT], out=True))
    with tile.TileContext(nc) as tc:
        with ExitStack() as ctx:
            _emit(ctx, tc, outs, ins)
    nc.compile()
    return nc


def host_prep(inputs):
    """Host-side layout marshalling: slice per core, add positional embeds,
    transpose to [feature, token], fold the attention scale into Wq
    (exact: 0.125 = 2^-3), pre-broadcast biases."""
    f32 = np.float32
    g = {k: np.asarray(v, dtype=f32) for k, v in inputs.items()}
    t_pat = g["t_x"][1:]                      # (VP, B*T, D)
    s_x = g["s_x"]                            # (AP, B*SPEC, D)

    posq = (g["vmae_space_pos"][:, None, :] + g["vmae_temporal_pos"][None, :, :])
    posq = posq.reshape(NQ, DIM)                                  # (NQ, D)
    poss = (g["clip_space_pos"][:, None, :] + g["clip_temporal_pos"][None, :, :])
    poss = poss.reshape(NK, DIM)                                  # (NK, D)

    wqT = np.ascontiguousarray((SCALE * g["Wq"]).T)
    wkT = np.ascontiguousarray(g["Wkv"][:DIM].T)
    wvT = np.ascontiguousarray(g["Wkv"][DIM:].T)
    wpT = np.ascontiguousarray(g["Wproj"].T)
    qb2 = np.ascontiguousarray((SCALE * g["q_bias"]).reshape(DC, 128).T)
    kb2 = np.ascontiguousarray(g["kv_bias"][:DIM].reshape(DC, 128).T)
    pb2 = np.ascontiguousarray(g["proj_bias"].reshape(DC, 128).T)
    vbb = np.ascontiguousarray(np.tile(g["kv_bias"][DIM:], (128, 1)))

    shared = dict(wqT=wqT, wkT=wkT, wvT=wvT, wpT=wpT,
                  qb2=qb2, kb2=kb2, pb2=pb2, vbb=vbb,
                  ones1=np.ones((1, HD), dtype=f32))
    in_maps = []
    for b in range(B):
        qin = t_pat[:, b * T:(b + 1) * T, :].reshape(NQ, DIM) + posq
        ss = s_x[:, b * SPEC:(b + 1) * SPEC, :].reshape(NK, DIM) + poss
        in_maps.append(dict(qinT=np.ascontiguousarray(qin.T),
                            ssT=np.ascontiguousarray(ss.T), **shared))
    return in_maps


def host_finish(results, t_x):
    # per-core out is [DIM, NQ] feature-major; transpose on host
    o = np.stack([results[b]["out"].T for b in range(B)])   # (B, NQ, D)
    o = o.reshape(B, VP, T, DIM).transpose(1, 0, 2, 3).reshape(VP, B * T, DIM)
    return np.concatenate([np.asarray(t_x, dtype=np.float32)[0:1], o], axis=0)


_NC = None


def kernel(**inputs):
    global _NC
    from concourse.bass_utils import run_bass_kernel_spmd
    if _NC is None:
        _NC = build_program()
    in_maps = host_prep(inputs)
    res = run_bass_kernel_spmd(_NC, in_maps, list(range(NCORES)))
    return host_finish(res.results, inputs["t_x"])
